# revision 14
# baseline (speedup 1.0000x reference)
"""AnomalyAttention Trainium2 kernel (8 NeuronCores, sequence-parallel).

Strategy: core c owns query rows [512c, 512c+512). x is passed in both
layouts (xT and natural, bf16) and replicated; no collectives (measured
intra-chip collective bandwidth is far below the cost of recomputing).

The matmul chains are reassociated so the big O(N^2 d) contractions run
against the raw x tensors and all weight applications stay O(N d^2) local:
  S   = Q K^T = (Qk^T)^T xT        with Qk^T = Wk^T Q^T   (tiny, local)
  S^T = (xT)^T Qk^T                (direct PE pass, no transposes)
  Z   = softmax(S) V = (Y^T)^T Wv^T with Y^T = x^T expS^T (4 PSUM groups)
This removes the full K^T and V projections entirely.

Big matmuls run in bf16 (fp32 matmul is 4x slower on the PE); sigma runs in
fp32. Softmax skips max-subtraction (logits are ~N(0,1)); exp and row-sum
fuse into one ScalarE pass via accum_out. The P prior depends only on sigma
and the iota row, so its whole pipeline (DVE: u, u^2; ScalarE: exp; DVE:
normalize) hides under the attention matmuls. P uses the exact denominator
sum(g) + 1e-8*sqrt(2*pi*sigma).
"""

import os
import sys

sys.path.insert(0, "/opt/trn_rl_repo")

import ml_dtypes
import numpy as np

import concourse.bass as bass
import concourse.mybir as mybir
import concourse.tile as tile
from concourse import bacc
from concourse.bass_utils import run_bass_kernel_spmd

F32 = mybir.dt.float32
BF16 = mybir.dt.bfloat16

N = 4096          # sequence length
D = 512           # model dim
CORES = 8
NL = N // CORES   # local query rows per core (512)
P128 = 128
QT = NL // P128   # 4 query tiles per core
FT = D // P128    # 4 feature tiles
DT = D // P128    # 4 output-dim tiles
NK = N // 512     # 8 key chunks of 512
NH = N // 1024    # 4 key chunks of 1024 (paired-PSUM exp)
KT = N // P128    # 32 key tiles of 128

SM_SCALE = 1.0 / float(np.sqrt(D))  # 1/sqrt(512)


def build():
    nc = bacc.Bacc("TRN2", target_bir_lowering=False, debug=False,
                   num_devices=CORES)

    xT = nc.dram_tensor("xT", [D, N], BF16, kind="ExternalInput").ap()
    xn = nc.dram_tensor("xn", [N, D], BF16, kind="ExternalInput").ap()
    xTl = nc.dram_tensor("xTl", [D, NL], F32, kind="ExternalInput").ap()
    xTlb = nc.dram_tensor("xTlb", [D, NL], BF16, kind="ExternalInput").ap()
    wqT = nc.dram_tensor("wqT", [D, D], BF16, kind="ExternalInput").ap()
    wk = nc.dram_tensor("wk", [D, D], BF16, kind="ExternalInput").ap()
    wvT = nc.dram_tensor("wvT", [D, D], BF16, kind="ExternalInput").ap()
    wsT = nc.dram_tensor("wsT", [D, 1], F32, kind="ExternalInput").ap()
    posr = nc.dram_tensor("posr", [1, N], F32, kind="ExternalInput").ap()
    posc = nc.dram_tensor("posc", [NL, 1], F32, kind="ExternalInput").ap()

    s_out = nc.dram_tensor("s_out", [NL, N], F32, kind="ExternalOutput").ap()
    p_out = nc.dram_tensor("p_out", [NL, N], F32, kind="ExternalOutput").ap()
    z_out = nc.dram_tensor("z_out", [NL, D], F32, kind="ExternalOutput").ap()

    xT3 = xT.rearrange("(ft p) n -> p ft n", p=P128)
    xn3 = xn.rearrange("(kt p) d -> p kt d", p=P128)

    with tile.TileContext(nc) as tc:
        with tc.tile_pool(name="persist", bufs=1) as per, \
             tc.tile_pool(name="stage", bufs=4) as stage, \
             tc.tile_pool(name="pp", bufs=1) as pp, \
             tc.tile_pool(name="gp", bufs=2) as gpool, \
             tc.tile_pool(name="pu", bufs=2) as pu, \
             tc.tile_pool(name="zop", bufs=2) as zop, \
             tc.tile_pool(name="vec", bufs=1) as vec:

            # ---- persistent SBUF tensors ----
            xt_bf = per.tile([P128, FT, N], BF16)      # x^T  [f, keys]
            xn_bf = per.tile([P128, KT, D], BF16)      # x    [keys, f]
            qT_bf = per.tile([P128, DT, NL], BF16)     # Q^T  [d, q]
            qkT_bf = per.tile([P128, FT, NL], BF16)    # (Q Wk)^T [f, q]
            ytT_bf = per.tile([P128, FT, NL], BF16)    # (expS x)^T [f, q]
            wvT_bf = per.tile([P128, FT, D], BF16)     # Wv^T [f, d]
            jb = pp.tile([P128, N], F32)               # broadcast iota row

            # small per-row vectors (columns indexed by q-tile)
            sig_c = vec.tile([P128, QT], F32)
            rsig_c = vec.tile([P128, QT], F32)
            extra_c = vec.tile([P128, QT], F32)
            negb_c = vec.tile([P128, QT], F32)
            posc_sb = vec.tile([P128, QT], F32)
            ws_sb = vec.tile([P128, FT], F32)
            rs_parts = vec.tile([P128, QT, NH], F32)   # S rowsum partials
            gp_parts = vec.tile([P128, QT, NK], F32)   # P gaussian sum partials
            rscale_c = vec.tile([P128, QT], F32)       # 1/rowsum(expS)
            rden_c = vec.tile([P128, QT], F32)         # 1/(sum g + eps term)
            tmp_c = vec.tile([P128, QT], F32)

            with tc.tile_pool(name="xw", bufs=1) as xw, \
                 tc.tile_pool(name="psA", bufs=2, space="PSUM") as psA:
                xtl_f = xw.tile([P128, FT, NL], F32)
                xtl_bf = xw.tile([P128, FT, NL], BF16)
                wq_bf = xw.tile([P128, FT, D], BF16)
                wk_bf = xw.tile([P128, DT, D], BF16)

                # ---- input DMAs ----
                # critical path first: xtl_bf + wq (-> Q), wk (-> Qk),
                # then xT chunks (-> S/S^T); everything else after.
                xtl_f_src = xTl.rearrange("(ft p) n -> p ft n", p=P128)
                xtl_b_src = xTlb.rearrange("(ft p) n -> p ft n", p=P128)
                wq_src = wqT.rearrange("(ft p) d -> p ft d", p=P128)
                wk_src = wk.rearrange("(dt p) f -> p dt f", p=P128)
                wv_src = wvT.rearrange("(ft p) d -> p ft d", p=P128)
                for ft in range(FT):
                    nc.sync.dma_start(out=xtl_bf[:, ft, :],
                                      in_=xtl_b_src[:, ft, :])
                    nc.sync.dma_start(out=wq_bf[:, ft, :],
                                      in_=wq_src[:, ft, :])
                for dt in range(DT):
                    nc.sync.dma_start(out=wk_bf[:, dt, :],
                                      in_=wk_src[:, dt, :])
                # xT / xn in key-chunk-major order (S/ST consume chunk 0 first)
                for nk in range(NK):
                    for ft in range(FT):
                        nc.sync.dma_start(
                            out=xt_bf[:, ft, nk * 512:(nk + 1) * 512],
                            in_=xT3[:, ft, nk * 512:(nk + 1) * 512])
                    for kt in range(nk * 4, nk * 4 + 4):
                        nc.scalar.dma_start(out=xn_bf[:, kt, :],
                                            in_=xn3[:, kt, :])
                nc.sync.dma_start(out=ws_sb[:, :],
                                  in_=wsT.rearrange("(t p) o -> p (t o)",
                                                    p=P128))
                nc.sync.dma_start(out=posc_sb[:, :],
                                  in_=posc.rearrange("(t p) o -> p (t o)",
                                                     p=P128))
                for ft in range(FT):
                    nc.sync.dma_start(out=xtl_f[:, ft, :],
                                      in_=xtl_f_src[:, ft, :])
                for ft in range(FT):
                    nc.sync.dma_start(out=wvT_bf[:, ft, :],
                                      in_=wv_src[:, ft, :])
                posr_b = bass.AP(tensor=posr.tensor, offset=posr.offset,
                                 ap=[[0, P128], [1, N]])
                nc.scalar.dma_start(out=jb[:, :], in_=posr_b)

                # ---- Q^T = Wq x_l^T ----
                for dt in range(DT):
                    q_ps = psA.tile([P128, NL], F32, tag="pj")
                    for ft in range(FT):
                        nc.tensor.matmul(
                            q_ps[:, :],
                            wq_bf[:, ft, dt * P128:(dt + 1) * P128],
                            xtl_bf[:, ft, :],
                            start=(ft == 0), stop=(ft == FT - 1))
                    nc.scalar.copy(out=qT_bf[:, dt, :], in_=q_ps[:, :])

                # ---- Qk^T = Wk^T Q^T ----
                for ft in range(FT):
                    qk_ps = psA.tile([P128, NL], F32, tag="pj")
                    for dt in range(DT):
                        nc.tensor.matmul(
                            qk_ps[:, :],
                            wk_bf[:, dt, ft * P128:(ft + 1) * P128],
                            qT_bf[:, dt, :],
                            start=(dt == 0), stop=(dt == DT - 1))
                    nc.scalar.copy(out=qkT_bf[:, ft, :], in_=qk_ps[:, :])

                # ---- sigma (fp32, natural layout [q, 1]) ----
                for qt in range(QT):
                    sg_ps = psA.tile([P128, 1], F32, tag="sg")
                    for ft in range(FT):
                        nc.tensor.matmul(
                            sg_ps[:, :],
                            xtl_f[:, ft, qt * P128:(qt + 1) * P128],
                            ws_sb[:, ft:ft + 1],
                            start=(ft == 0), stop=(ft == FT - 1))
                    nc.vector.tensor_scalar_max(
                        out=sig_c[:, qt:qt + 1], in0=sg_ps[:, :], scalar1=0.001)
                nc.vector.tensor_scalar_min(
                    out=sig_c[:, :], in0=sig_c[:, :], scalar1=1.0)
                nc.vector.reciprocal(out=rsig_c[:, :], in_=sig_c[:, :])
                # 1e-8*sqrt(2*pi*sigma) = sqrt(sigma * 2*pi*1e-16)
                nc.scalar.activation(
                    out=extra_c[:, :], in_=sig_c[:, :],
                    func=mybir.ActivationFunctionType.Sqrt,
                    scale=float(2.0 * np.pi * 1e-16))
                nc.vector.tensor_mul(
                    out=tmp_c[:, :], in0=posc_sb[:, :], in1=rsig_c[:, :])
                nc.vector.tensor_scalar_mul(
                    out=negb_c[:, :], in0=tmp_c[:, :], scalar1=-1.0)


            # ---- S and S^T ----
            with tc.tile_pool(name="late", bufs=1) as late:
                exps_bf = late.tile([P128, QT, N], BF16)   # exp(S*sc) [q, keys]
                expsT_bf = late.tile([P128, KT, NL], BF16)  # same, [keys, q]

                with tc.tile_pool(name="psS", bufs=2, space="PSUM") as psS, \
                     tc.tile_pool(name="psT", bufs=3, space="PSUM") as psT:
                    # S matmuls + fused exp/rowsum (paired 1024-wide PSUM)
                    for qt in range(QT):
                        for nh in range(NH):
                            s_ps = psS.tile([P128, 2, 512], F32, tag="s")
                            for half in range(2):
                                nk = nh * 2 + half
                                for ft in range(FT):
                                    nc.tensor.matmul(
                                        s_ps[:, half, :],
                                        qkT_bf[:, ft,
                                               qt * P128:(qt + 1) * P128],
                                        xt_bf[:, ft, nk * 512:(nk + 1) * 512],
                                        start=(ft == 0), stop=(ft == FT - 1))
                            nc.scalar.activation(
                                out=exps_bf[:, qt, nh * 1024:(nh + 1) * 1024],
                                in_=s_ps[:, :, :],
                                func=mybir.ActivationFunctionType.Exp,
                                scale=SM_SCALE,
                                accum_out=rs_parts[:, qt, nh:nh + 1])

                    # row scales
                    for qt in range(QT):
                        nc.vector.reduce_sum(
                            out=tmp_c[:, qt:qt + 1], in_=rs_parts[:, qt, :],
                            axis=mybir.AxisListType.X)
                    nc.vector.reciprocal(out=rscale_c[:, :], in_=tmp_c[:, :])

                    # S output (scaled exp)
                    for qt in range(QT):
                        for nk in range(NK):
                            so = stage.tile([P128, 512], F32, tag="so")
                            nc.vector.tensor_scalar_mul(
                                out=so[:, :],
                                in0=exps_bf[:, qt, nk * 512:(nk + 1) * 512],
                                scalar1=rscale_c[:, qt:qt + 1])
                            nc.sync.dma_start(
                                out=s_out[qt * P128:(qt + 1) * P128,
                                          nk * 512:(nk + 1) * 512],
                                in_=so[:, :])

                    # S^T = (xT)^T Qk^T directly on the PE (exp'd, unscaled)
                    for kt in range(KT):
                        st_ps = psT.tile([P128, NL], F32, tag="st")
                        for ft in range(FT):
                            nc.tensor.matmul(
                                st_ps[:, :],
                                xt_bf[:, ft, kt * P128:(kt + 1) * P128],
                                qkT_bf[:, ft, :],
                                start=(ft == 0), stop=(ft == FT - 1))
                        nc.scalar.activation(
                            out=expsT_bf[:, kt, :], in_=st_ps[:, :],
                            func=mybir.ActivationFunctionType.Exp,
                            scale=SM_SCALE)

                # ---- P prior association (independent of attention — hides
                # under the S/S^T matmuls; DVE: u, u^2; ACT: exp; DVE: out)
                for qt in range(QT):
                    g_bf = gpool.tile([P128, N], BF16, tag="g")
                    for nk in range(NK):
                        u = pu.tile([P128, 512], F32, tag="u")
                        nc.vector.tensor_scalar(
                            out=u[:, :], in0=jb[:, nk * 512:(nk + 1) * 512],
                            scalar1=rsig_c[:, qt:qt + 1],
                            scalar2=negb_c[:, qt:qt + 1],
                            op0=mybir.AluOpType.mult,
                            op1=mybir.AluOpType.add)
                        nc.vector.tensor_mul(out=u[:, :], in0=u[:, :],
                                             in1=u[:, :])
                        nc.scalar.activation(
                            out=g_bf[:, nk * 512:(nk + 1) * 512],
                            in_=u[:, :],
                            func=mybir.ActivationFunctionType.Exp,
                            scale=-0.5,
                            accum_out=gp_parts[:, qt, nk:nk + 1])
                    nc.vector.reduce_sum(
                        out=tmp_c[:, qt:qt + 1], in_=gp_parts[:, qt, :],
                        axis=mybir.AxisListType.X)
                    nc.vector.tensor_add(
                        out=tmp_c[:, qt:qt + 1], in0=tmp_c[:, qt:qt + 1],
                        in1=extra_c[:, qt:qt + 1])
                    nc.vector.reciprocal(
                        out=rden_c[:, qt:qt + 1], in_=tmp_c[:, qt:qt + 1])
                    for nk in range(NK):
                        po = stage.tile([P128, 512], F32, tag="po")
                        nc.vector.tensor_scalar_mul(
                            out=po[:, :],
                            in0=g_bf[:, nk * 512:(nk + 1) * 512],
                            scalar1=rden_c[:, qt:qt + 1])
                        nc.gpsimd.dma_start(
                            out=p_out[qt * P128:(qt + 1) * P128,
                                      nk * 512:(nk + 1) * 512],
                            in_=po[:, :])

                # ---- Y^T = x^T expS^T, then Z = (Y^T)^T Wv^T ----
                with tc.tile_pool(name="psY", bufs=2, space="PSUM") as psY, \
                     tc.tile_pool(name="psZ", bufs=2, space="PSUM") as psZ:
                    for ft in range(FT):
                        yt_ps = psY.tile([P128, NL], F32, tag="yt")
                        for kt in range(KT):
                            nc.tensor.matmul(
                                yt_ps[:, :],
                                xn_bf[:, kt, ft * P128:(ft + 1) * P128],
                                expsT_bf[:, kt, :],
                                start=(kt == 0), stop=(kt == KT - 1))
                        nc.scalar.copy(out=ytT_bf[:, ft, :], in_=yt_ps[:, :])

                    for qt in range(QT):
                        z_ps = psZ.tile([P128, D], F32, tag="z")
                        for ft in range(FT):
                            nc.tensor.matmul(
                                z_ps[:, :],
                                ytT_bf[:, ft, qt * P128:(qt + 1) * P128],
                                wvT_bf[:, ft, :],
                                start=(ft == 0), stop=(ft == FT - 1))
                        zo = zop.tile([P128, D], F32, tag="zo")
                        nc.vector.tensor_scalar_mul(
                            out=zo[:, :], in0=z_ps[:, :],
                            scalar1=rscale_c[:, qt:qt + 1])
                        nc.sync.dma_start(
                            out=z_out[qt * P128:(qt + 1) * P128, :],
                            in_=zo[:, :])

    nc.compile()
    return nc


_CACHE = {}


def _get_nc():
    if "nc" not in _CACHE:
        _CACHE["nc"] = build()
    return _CACHE["nc"]


def _make_in_maps(x, Wq, Wk, Wv, Ws):
    x = np.asarray(x, dtype=np.float32)
    Wq = np.asarray(Wq, dtype=np.float32)
    Wk = np.asarray(Wk, dtype=np.float32)
    Wv = np.asarray(Wv, dtype=np.float32)
    Ws = np.asarray(Ws, dtype=np.float32)

    bf = ml_dtypes.bfloat16
    xT = np.ascontiguousarray(x.T)                 # [D, N] f32
    xT_b = xT.astype(bf)
    xn_b = x.astype(bf)
    wqT = np.ascontiguousarray(Wq.T).astype(bf)    # [in, out]
    wk_b = np.ascontiguousarray(Wk).astype(bf)     # [out, in] (natural)
    wvT = np.ascontiguousarray(Wv.T).astype(bf)
    wsT = np.ascontiguousarray(Ws.T)               # [D, 1] f32
    pos = np.arange(N, dtype=np.float32)

    in_maps = []
    for c in range(CORES):
        in_maps.append({
            "xT": xT_b,
            "xn": xn_b,
            "xTl": np.ascontiguousarray(xT[:, c * NL:(c + 1) * NL]),
            "xTlb": np.ascontiguousarray(xT_b[:, c * NL:(c + 1) * NL]),
            "wqT": wqT, "wk": wk_b, "wvT": wvT, "wsT": wsT,
            "posr": pos[None, :],
            "posc": np.ascontiguousarray(pos[c * NL:(c + 1) * NL, None]),
        })
    return in_maps


def _gather(results):
    Z = np.concatenate([r["z_out"] for r in results], axis=0)
    P = np.concatenate([r["p_out"] for r in results], axis=0)
    S = np.concatenate([r["s_out"] for r in results], axis=0)
    return Z, P, S


def _ensure_ntff_hook():
    """The agent image's antenv lacks axon_hooks; build the NTFF profiling
    hook from the injected libaxon .so (same mechanism as trn_boot)."""
    try:
        from antenv.axon_hooks import get_axon_ntff_profile_hook  # noqa: F401
        return
    except ImportError:
        pass
    import types

    import antenv

    if "/root/.axon_site" not in sys.path:
        sys.path.insert(0, "/root/.axon_site")
    from trn_agent_boot.trn_boot import _ntff_profile_via_ctypes

    hook = _ntff_profile_via_ctypes("/opt/axon/libaxon_pjrt.so")
    mod = types.ModuleType("antenv.axon_hooks")
    state = {"hook": hook}
    mod.get_axon_ntff_profile_hook = lambda: state["hook"]
    mod.set_axon_ntff_profile_hook = lambda h: state.__setitem__("hook", h)
    sys.modules["antenv.axon_hooks"] = mod
    antenv.axon_hooks = mod


def run(x, Wq, Wk, Wv, Ws, trace=False):
    if trace:
        _ensure_ntff_hook()
    nc = _get_nc()
    in_maps = _make_in_maps(x, Wq, Wk, Wv, Ws)
    res = run_bass_kernel_spmd(nc, in_maps, list(range(CORES)), trace=trace)
    return _gather(res.results), res


def kernel(x, Wq, Wk, Wv, Ws):
    (Z, P, S), _ = run(x, Wq, Wk, Wv, Ws,
                       trace=bool(int(os.environ.get("KERNEL_TRACE", "0"))))
    return Z, P, S


# revision 19
# speedup vs baseline: 1.1855x; 1.1855x over previous
"""AnomalyAttention Trainium2 kernel (8 NeuronCores, sequence-parallel).

Strategy: core c owns query rows [512c, 512c+512). x is passed in both
layouts (xT and natural, bf16) and replicated; no collectives (measured
intra-chip collective bandwidth is far below the cost of recomputing).

The matmul chains are reassociated so the big O(N^2 d) contractions run
against the raw x tensors and all weight applications stay O(N d^2) local:
  S   = Q K^T = (Qk^T)^T xT        with Qk^T = Wk^T Q^T   (tiny, local)
  S^T = (xT)^T Qk^T                (direct PE pass, no transposes)
  Z   = softmax(S) V = (Y^T)^T Wv^T with Y^T = x^T expS^T (4 PSUM groups)
This removes the full K^T and V projections entirely.

Big matmuls run in bf16 (fp32 matmul is 4x slower on the PE); sigma runs in
fp32. Softmax skips max-subtraction (logits are ~N(0,1)); exp and row-sum
fuse into one ScalarE pass via accum_out. The P prior depends only on sigma
and the iota row, so its whole pipeline (DVE: u, u^2; ScalarE: exp; DVE:
normalize) hides under the attention matmuls. P uses the exact denominator
sum(g) + 1e-8*sqrt(2*pi*sigma).
"""

import os
import sys

sys.path.insert(0, "/opt/trn_rl_repo")

import ml_dtypes
import numpy as np

import concourse.bass as bass
import concourse.mybir as mybir
import concourse.tile as tile
from concourse import bacc
from concourse.bass_utils import run_bass_kernel_spmd

F32 = mybir.dt.float32
BF16 = mybir.dt.bfloat16

N = 4096          # sequence length
D = 512           # model dim
CORES = 8
NL = N // CORES   # local query rows per core (512)
P128 = 128
QT = NL // P128   # 4 query tiles per core
FT = D // P128    # 4 feature tiles
DT = D // P128    # 4 output-dim tiles
NK = N // 512     # 8 key chunks of 512
NH = N // 1024    # 4 key chunks of 1024 (paired-PSUM exp)
KT = N // P128    # 32 key tiles of 128

SM_SCALE = 1.0 / float(np.sqrt(D))  # 1/sqrt(512)


def build():
    nc = bacc.Bacc("TRN2", target_bir_lowering=False, debug=False,
                   num_devices=CORES)

    xT = nc.dram_tensor("xT", [D, N], BF16, kind="ExternalInput").ap()
    xn = nc.dram_tensor("xn", [N, D], BF16, kind="ExternalInput").ap()
    xTl = nc.dram_tensor("xTl", [D, NL], F32, kind="ExternalInput").ap()
    xTlb = nc.dram_tensor("xTlb", [D, NL], BF16, kind="ExternalInput").ap()
    wqT = nc.dram_tensor("wqT", [D, D], BF16, kind="ExternalInput").ap()
    wk = nc.dram_tensor("wk", [D, D], BF16, kind="ExternalInput").ap()
    wvT = nc.dram_tensor("wvT", [D, D], BF16, kind="ExternalInput").ap()
    wsT = nc.dram_tensor("wsT", [D, 1], F32, kind="ExternalInput").ap()
    posr = nc.dram_tensor("posr", [1, N], F32, kind="ExternalInput").ap()
    posc = nc.dram_tensor("posc", [NL, 1], F32, kind="ExternalInput").ap()

    s_out = nc.dram_tensor("s_out", [NL, N], F32, kind="ExternalOutput").ap()
    p_out = nc.dram_tensor("p_out", [NL, N], F32, kind="ExternalOutput").ap()
    z_out = nc.dram_tensor("z_out", [NL, D], F32, kind="ExternalOutput").ap()

    xT3 = xT.rearrange("(ft p) n -> p ft n", p=P128)
    xn3 = xn.rearrange("(kt p) d -> p kt d", p=P128)

    with tile.TileContext(nc) as tc:
        with tc.tile_pool(name="persist", bufs=1) as per, \
             tc.tile_pool(name="stage", bufs=2) as stage, \
             tc.tile_pool(name="pp", bufs=1) as pp, \
             tc.tile_pool(name="gp", bufs=2) as gpool, \
             tc.tile_pool(name="pu", bufs=2) as pu, \
             tc.tile_pool(name="zop", bufs=2) as zop, \
             tc.tile_pool(name="vec", bufs=1) as vec:

            # ---- persistent SBUF tensors ----
            xt_bf = per.tile([P128, FT, N], BF16)      # x^T  [f, keys]
            xn_bf = per.tile([P128, KT, D], BF16)      # x    [keys, f]
            qT_bf = per.tile([P128, DT, NL], BF16)     # Q^T  [d, q]
            qkT_bf = per.tile([P128, FT, NL], BF16)    # (Q Wk)^T [f, q]
            ytT_bf = per.tile([P128, FT, NL], BF16)    # (expS x)^T [f, q]
            wvT_bf = per.tile([P128, FT, D], BF16)     # Wv^T [f, d]
            jb = pp.tile([P128, N], F32)               # broadcast iota row

            # small per-row vectors (columns indexed by q-tile)
            sig_c = vec.tile([P128, QT], F32)
            rsig_c = vec.tile([P128, QT], F32)
            extra_c = vec.tile([P128, QT], F32)
            negb_c = vec.tile([P128, QT], F32)
            posc_sb = vec.tile([P128, QT], F32)
            ws_sb = vec.tile([P128, FT], F32)
            rs_parts = vec.tile([P128, QT, NH], F32)   # S rowsum partials
            gp_parts = vec.tile([P128, QT, NK], F32)   # P gaussian sum partials
            rscale_c = vec.tile([P128, QT], F32)       # 1/rowsum(expS)
            rden_c = vec.tile([P128, QT], F32)         # 1/(sum g + eps term)
            tmp_c = vec.tile([P128, QT], F32)

            with tc.tile_pool(name="xw", bufs=1) as xw, \
                 tc.tile_pool(name="psA", bufs=2, space="PSUM") as psA:
                xtl_f = xw.tile([P128, FT, NL], F32)
                xtl_bf = xw.tile([P128, FT, NL], BF16)
                wq_bf = xw.tile([P128, FT, D], BF16)
                wk_bf = xw.tile([P128, DT, D], BF16)

                # ---- input DMAs ----
                # Issue bandwidth matters (~1us per dma_start on a
                # sequencer): few, large DMAs, split across the three
                # issuers (sync, scalar=ACT HWDGE, gpsimd SWDGE).
                # sync: critical path  xtl_bf+wq -> Q, wk -> Qk, xT -> S/S^T
                xtl_f_src = xTl.rearrange("(ft p) n -> p ft n", p=P128)
                xtl_b_src = xTlb.rearrange("(ft p) n -> p ft n", p=P128)
                wq_src = wqT.rearrange("(ft p) d -> p ft d", p=P128)
                wk_src = wk.rearrange("(dt p) f -> p dt f", p=P128)
                wv_src = wvT.rearrange("(ft p) d -> p ft d", p=P128)
                for ft in range(FT):
                    nc.sync.dma_start(out=xtl_bf[:, ft, :],
                                      in_=xtl_b_src[:, ft, :])
                    nc.sync.dma_start(out=wq_bf[:, ft, :],
                                      in_=wq_src[:, ft, :])
                for dt in range(DT):
                    nc.sync.dma_start(out=wk_bf[:, dt, :],
                                      in_=wk_src[:, dt, :])
                # xT: first two key chunks fine-grained (latency), rest coarse
                for nk in range(2):
                    for ft in range(FT):
                        nc.sync.dma_start(
                            out=xt_bf[:, ft, nk * 512:(nk + 1) * 512],
                            in_=xT3[:, ft, nk * 512:(nk + 1) * 512])
                for nk in range(2, NK):
                    nc.sync.dma_start(
                        out=xt_bf[:, :, nk * 512:(nk + 1) * 512],
                        in_=xT3[:, :, nk * 512:(nk + 1) * 512])
                # scalar: sigma inputs early, then wv
                for ft in range(FT):
                    nc.scalar.dma_start(out=xtl_f[:, ft, :],
                                        in_=xtl_f_src[:, ft, :])
                nc.scalar.dma_start(out=ws_sb[:, :],
                                    in_=wsT.rearrange("(t p) o -> p (t o)",
                                                      p=P128))
                nc.scalar.dma_start(out=posc_sb[:, :],
                                    in_=posc.rearrange("(t p) o -> p (t o)",
                                                       p=P128))
                for ft in range(FT):
                    nc.scalar.dma_start(out=wvT_bf[:, ft, :],
                                        in_=wv_src[:, ft, :])
                posr_b = bass.AP(tensor=posr.tensor, offset=posr.offset,
                                 ap=[[0, P128], [1, N]])
                nc.scalar.dma_start(out=jb[:, :], in_=posr_b)
                # gpsimd: xn (only needed by Y^T, late)
                for h in range(4):
                    nc.gpsimd.dma_start(
                        out=xn_bf[:, h * 8:(h + 1) * 8, :],
                        in_=xn3[:, h * 8:(h + 1) * 8, :])

                # ---- Q^T = Wq x_l^T ----
                for dt in range(DT):
                    q_ps = psA.tile([P128, NL], F32, tag="pj")
                    for ft in range(FT):
                        nc.tensor.matmul(
                            q_ps[:, :],
                            wq_bf[:, ft, dt * P128:(dt + 1) * P128],
                            xtl_bf[:, ft, :],
                            start=(ft == 0), stop=(ft == FT - 1))
                    nc.scalar.copy(out=qT_bf[:, dt, :], in_=q_ps[:, :])

                # ---- Qk^T = Wk^T Q^T ----
                for ft in range(FT):
                    qk_ps = psA.tile([P128, NL], F32, tag="pj")
                    for dt in range(DT):
                        nc.tensor.matmul(
                            qk_ps[:, :],
                            wk_bf[:, dt, ft * P128:(ft + 1) * P128],
                            qT_bf[:, dt, :],
                            start=(dt == 0), stop=(dt == DT - 1))
                    nc.scalar.copy(out=qkT_bf[:, ft, :], in_=qk_ps[:, :])

                # ---- sigma (fp32, natural layout [q, 1]) ----
                for qt in range(QT):
                    sg_ps = psA.tile([P128, 1], F32, tag="sg")
                    for ft in range(FT):
                        nc.tensor.matmul(
                            sg_ps[:, :],
                            xtl_f[:, ft, qt * P128:(qt + 1) * P128],
                            ws_sb[:, ft:ft + 1],
                            start=(ft == 0), stop=(ft == FT - 1))
                    nc.vector.tensor_scalar_max(
                        out=sig_c[:, qt:qt + 1], in0=sg_ps[:, :], scalar1=0.001)
                nc.vector.tensor_scalar_min(
                    out=sig_c[:, :], in0=sig_c[:, :], scalar1=1.0)
                nc.vector.reciprocal(out=rsig_c[:, :], in_=sig_c[:, :])
                # 1e-8*sqrt(2*pi*sigma) = sqrt(sigma * 2*pi*1e-16)
                nc.scalar.activation(
                    out=extra_c[:, :], in_=sig_c[:, :],
                    func=mybir.ActivationFunctionType.Sqrt,
                    scale=float(2.0 * np.pi * 1e-16))
                nc.vector.tensor_mul(
                    out=tmp_c[:, :], in0=posc_sb[:, :], in1=rsig_c[:, :])
                nc.vector.tensor_scalar_mul(
                    out=negb_c[:, :], in0=tmp_c[:, :], scalar1=-1.0)


            # ---- S and S^T ----
            with tc.tile_pool(name="late", bufs=1) as late:
                exps_bf = late.tile([P128, QT, N], BF16)   # exp(S*sc) [q, keys]
                expsT_bf = late.tile([P128, KT, NL], BF16)  # same, [keys, q]

                psS = tc.alloc_tile_pool(name="psS", bufs=2, space="PSUM")
                psT = tc.alloc_tile_pool(name="psT", bufs=3, space="PSUM")
                if True:
                    # S matmuls + fused exp/rowsum (paired 1024-wide PSUM)
                    for qt in range(QT):
                        for nh in range(NH):
                            s_ps = psS.tile([P128, 2, 512], F32, tag="s")
                            for half in range(2):
                                nk = nh * 2 + half
                                for ft in range(FT):
                                    nc.tensor.matmul(
                                        s_ps[:, half, :],
                                        qkT_bf[:, ft,
                                               qt * P128:(qt + 1) * P128],
                                        xt_bf[:, ft, nk * 512:(nk + 1) * 512],
                                        start=(ft == 0), stop=(ft == FT - 1))
                            nc.scalar.activation(
                                out=exps_bf[:, qt, nh * 1024:(nh + 1) * 1024],
                                in_=s_ps[:, :, :],
                                func=mybir.ActivationFunctionType.Exp,
                                scale=SM_SCALE,
                                accum_out=rs_parts[:, qt, nh:nh + 1])

                    # row scales
                    for qt in range(QT):
                        nc.vector.reduce_sum(
                            out=tmp_c[:, qt:qt + 1], in_=rs_parts[:, qt, :],
                            axis=mybir.AxisListType.X)
                    nc.vector.reciprocal(out=rscale_c[:, :], in_=tmp_c[:, :])

                    # S output (scaled exp, bf16-staged; SWDGE casts
                    # to f32 on the way out)
                    for qt in range(QT):
                        for c in range(2):
                            so = stage.tile([P128, 2048], BF16, tag="so")
                            nc.vector.tensor_scalar_mul(
                                out=so[:, :],
                                in0=exps_bf[:, qt, c * 2048:(c + 1) * 2048],
                                scalar1=rscale_c[:, qt:qt + 1])
                            nc.gpsimd.dma_start(
                                out=s_out[qt * P128:(qt + 1) * P128,
                                          c * 2048:(c + 1) * 2048],
                                in_=so[:, :])

                    # S^T = (xT)^T Qk^T directly on the PE (exp'd, unscaled)
                    for kt in range(KT):
                        st_ps = psT.tile([P128, NL], F32, tag="st")
                        for ft in range(FT):
                            nc.tensor.matmul(
                                st_ps[:, :],
                                xt_bf[:, ft, kt * P128:(kt + 1) * P128],
                                qkT_bf[:, ft, :],
                                start=(ft == 0), stop=(ft == FT - 1))
                        nc.scalar.activation(
                            out=expsT_bf[:, kt, :], in_=st_ps[:, :],
                            func=mybir.ActivationFunctionType.Exp,
                            scale=SM_SCALE)

                # ---- P prior association (independent of attention — hides
                # under the S/S^T matmuls; DVE: u, u^2; ACT: exp; DVE: out)
                for qt in range(QT):
                    g_bf = gpool.tile([P128, N], BF16, tag="g")
                    for nk in range(NK):
                        u = pu.tile([P128, 512], F32, tag="u")
                        nc.vector.tensor_scalar(
                            out=u[:, :], in0=jb[:, nk * 512:(nk + 1) * 512],
                            scalar1=rsig_c[:, qt:qt + 1],
                            scalar2=negb_c[:, qt:qt + 1],
                            op0=mybir.AluOpType.mult,
                            op1=mybir.AluOpType.add)
                        nc.vector.tensor_mul(out=u[:, :], in0=u[:, :],
                                             in1=u[:, :])
                        nc.scalar.activation(
                            out=g_bf[:, nk * 512:(nk + 1) * 512],
                            in_=u[:, :],
                            func=mybir.ActivationFunctionType.Exp,
                            scale=-0.5,
                            accum_out=gp_parts[:, qt, nk:nk + 1])
                    nc.vector.reduce_sum(
                        out=tmp_c[:, qt:qt + 1], in_=gp_parts[:, qt, :],
                        axis=mybir.AxisListType.X)
                    nc.vector.tensor_add(
                        out=tmp_c[:, qt:qt + 1], in0=tmp_c[:, qt:qt + 1],
                        in1=extra_c[:, qt:qt + 1])
                    nc.vector.reciprocal(
                        out=rden_c[:, qt:qt + 1], in_=tmp_c[:, qt:qt + 1])
                    for c in range(2):
                        po = stage.tile([P128, 2048], BF16, tag="po")
                        nc.vector.tensor_scalar_mul(
                            out=po[:, :],
                            in0=g_bf[:, c * 2048:(c + 1) * 2048],
                            scalar1=rden_c[:, qt:qt + 1])
                        nc.gpsimd.dma_start(
                            out=p_out[qt * P128:(qt + 1) * P128,
                                      c * 2048:(c + 1) * 2048],
                            in_=po[:, :])

                # ---- Y^T = x^T expS^T, then Z = (Y^T)^T Wv^T ----
                if True:
                    for ft in range(FT):
                        yt_ps = psS.tile([P128, NL], F32, tag="s")
                        for kt in range(KT):
                            nc.tensor.matmul(
                                yt_ps[:, :],
                                xn_bf[:, kt, ft * P128:(ft + 1) * P128],
                                expsT_bf[:, kt, :],
                                start=(kt == 0), stop=(kt == KT - 1))
                        nc.scalar.copy(out=ytT_bf[:, ft, :], in_=yt_ps[:, :])

                    for qt in range(QT):
                        z_ps = psT.tile([P128, D], F32, tag="st")
                        for ft in range(FT):
                            nc.tensor.matmul(
                                z_ps[:, :],
                                ytT_bf[:, ft, qt * P128:(qt + 1) * P128],
                                wvT_bf[:, ft, :],
                                start=(ft == 0), stop=(ft == FT - 1))
                        zo = zop.tile([P128, D], F32, tag="zo")
                        nc.vector.tensor_scalar_mul(
                            out=zo[:, :], in0=z_ps[:, :],
                            scalar1=rscale_c[:, qt:qt + 1])
                        nc.sync.dma_start(
                            out=z_out[qt * P128:(qt + 1) * P128, :],
                            in_=zo[:, :])
                psT.release()
                psS.release()

    nc.compile()
    return nc


_CACHE = {}


def _get_nc():
    if "nc" not in _CACHE:
        _CACHE["nc"] = build()
    return _CACHE["nc"]


def _make_in_maps(x, Wq, Wk, Wv, Ws):
    x = np.asarray(x, dtype=np.float32)
    Wq = np.asarray(Wq, dtype=np.float32)
    Wk = np.asarray(Wk, dtype=np.float32)
    Wv = np.asarray(Wv, dtype=np.float32)
    Ws = np.asarray(Ws, dtype=np.float32)

    bf = ml_dtypes.bfloat16
    xT = np.ascontiguousarray(x.T)                 # [D, N] f32
    xT_b = xT.astype(bf)
    xn_b = x.astype(bf)
    wqT = np.ascontiguousarray(Wq.T).astype(bf)    # [in, out]
    wk_b = np.ascontiguousarray(Wk).astype(bf)     # [out, in] (natural)
    wvT = np.ascontiguousarray(Wv.T).astype(bf)
    wsT = np.ascontiguousarray(Ws.T)               # [D, 1] f32
    pos = np.arange(N, dtype=np.float32)

    in_maps = []
    for c in range(CORES):
        in_maps.append({
            "xT": xT_b,
            "xn": xn_b,
            "xTl": np.ascontiguousarray(xT[:, c * NL:(c + 1) * NL]),
            "xTlb": np.ascontiguousarray(xT_b[:, c * NL:(c + 1) * NL]),
            "wqT": wqT, "wk": wk_b, "wvT": wvT, "wsT": wsT,
            "posr": pos[None, :],
            "posc": np.ascontiguousarray(pos[c * NL:(c + 1) * NL, None]),
        })
    return in_maps


def _gather(results):
    Z = np.concatenate([r["z_out"] for r in results], axis=0)
    P = np.concatenate([r["p_out"] for r in results], axis=0)
    S = np.concatenate([r["s_out"] for r in results], axis=0)
    return Z, P, S


def _ensure_ntff_hook():
    """The agent image's antenv lacks axon_hooks; build the NTFF profiling
    hook from the injected libaxon .so (same mechanism as trn_boot)."""
    try:
        from antenv.axon_hooks import get_axon_ntff_profile_hook  # noqa: F401
        return
    except ImportError:
        pass
    import types

    import antenv

    if "/root/.axon_site" not in sys.path:
        sys.path.insert(0, "/root/.axon_site")
    from trn_agent_boot.trn_boot import _ntff_profile_via_ctypes

    hook = _ntff_profile_via_ctypes("/opt/axon/libaxon_pjrt.so")
    mod = types.ModuleType("antenv.axon_hooks")
    state = {"hook": hook}
    mod.get_axon_ntff_profile_hook = lambda: state["hook"]
    mod.set_axon_ntff_profile_hook = lambda h: state.__setitem__("hook", h)
    sys.modules["antenv.axon_hooks"] = mod
    antenv.axon_hooks = mod


def run(x, Wq, Wk, Wv, Ws, trace=False):
    if trace:
        _ensure_ntff_hook()
    nc = _get_nc()
    in_maps = _make_in_maps(x, Wq, Wk, Wv, Ws)
    res = run_bass_kernel_spmd(nc, in_maps, list(range(CORES)), trace=trace)
    return _gather(res.results), res


def kernel(x, Wq, Wk, Wv, Ws):
    (Z, P, S), _ = run(x, Wq, Wk, Wv, Ws,
                       trace=bool(int(os.environ.get("KERNEL_TRACE", "0"))))
    return Z, P, S
